# revision 7
# baseline (speedup 1.0000x reference)
"""Trainium2 Bass kernel for the contrastive loss problem.

Math reformulation of the reference (no [N, 2N-1] scatter needed):
  lse_i = log( exp(pos_val_i) + sum_{j in neg} exp(S_ij) + (2N-2-num_neg_i) )
  loss  = mean_i (lse_i - pos_val_i)
with S = (cos + 1) * 0.25, cos from row-normalized embeddings.

Sharding: data parallel over rows. Each of the 8 cores computes its
[512, 4096] block of exp(S) via bf16 matmul against the full normalized
embedding matrix (d-major in SBUF), does the label-masked row sums on
ScalarE/VectorE, and returns 512 per-row losses. Host does the final sum.

Host-side prep (cheap, O(N*D) or label-only): row norms, bf16 cast,
transpose to d-major, first-positive index per row (pure label metadata),
and the negatives count.
"""

import sys

sys.path.insert(0, "/opt/trn_rl_repo")

from contextlib import ExitStack

import ml_dtypes
import numpy as np

import concourse.bacc as bacc
import concourse.tile as tile
from concourse import mybir
from concourse.bass_utils import run_bass_kernel_spmd

N, D = 4096, 1024
NCORES = 8
R = N // NCORES            # 512 rows per core
P = 128                    # partitions
MI = R // P                # 4 row chunks per core
KC = D // P                # 8 contraction chunks
JW = 512                   # j tile width (one PSUM bank)
JB = N // JW               # 8 j tiles
EPS = 1e-8
BF16 = ml_dtypes.bfloat16

_CACHE = {}


def _build_program():
    nc = bacc.Bacc("TRN2", target_bir_lowering=False, debug=False)
    f32, bf16 = mybir.dt.float32, mybir.dt.bfloat16
    AF = mybir.ActivationFunctionType
    OP = mybir.AluOpType

    et_d = nc.dram_tensor("et", [KC, P, N], bf16, kind="ExternalInput")
    ebt_d = nc.dram_tensor("ebt", [KC, P, R], bf16, kind="ExternalInput")
    yt_d = nc.dram_tensor("yt", [P, N], bf16, kind="ExternalInput")
    yb_d = nc.dram_tensor("yb", [P, MI], f32, kind="ExternalInput")
    en_d = nc.dram_tensor("en", [MI, P, D], bf16, kind="ExternalInput")
    ef_d = nc.dram_tensor("ef", [MI, P, D], bf16, kind="ExternalInput")
    hp_d = nc.dram_tensor("hp", [P, MI], f32, kind="ExternalInput")
    zc_d = nc.dram_tensor("zc", [P, MI], f32, kind="ExternalInput")
    out_d = nc.dram_tensor("partial", [P, MI], f32, kind="ExternalOutput")

    with tile.TileContext(nc) as tc, ExitStack() as ctx:
        const = ctx.enter_context(tc.tile_pool(name="const", bufs=1))
        psum = ctx.enter_context(tc.tile_pool(name="psum", bufs=4, space="PSUM"))
        work = ctx.enter_context(tc.tile_pool(name="work", bufs=3))
        acc = ctx.enter_context(tc.tile_pool(name="acc", bufs=2))

        et = const.tile([P, KC, N], bf16, tag="et")
        ebt = const.tile([P, KC, R], bf16, tag="ebt")
        yt = const.tile([P, N], bf16, tag="yt")
        yb = const.tile([P, MI], f32, tag="yb")
        en = const.tile([P, MI, D], bf16, tag="en")
        ef = const.tile([P, MI, D], bf16, tag="ef")
        hp = const.tile([P, MI], f32, tag="hp")
        zc = const.tile([P, MI], f32, tag="zc")
        dummy = const.tile([P, 1], f32, tag="dummy")
        b025 = const.tile([P, 1], f32, tag="b025")
        nc.vector.memset(b025, 0.25)
        tot = const.tile([P, MI], f32, tag="tot")
        ps = const.tile([P, MI], f32, tag="ps")
        pv = const.tile([P, MI], f32, tag="pv")
        lse = const.tile([P, MI], f32, tag="lse")
        rl = const.tile([P, MI], f32, tag="rl")

        for k in range(KC):
            nc.sync.dma_start(out=et[:, k, :], in_=et_d[k])
            nc.sync.dma_start(out=ebt[:, k, :], in_=ebt_d[k])
        nc.sync.dma_start(out=yt, in_=yt_d[:])
        nc.sync.dma_start(out=yb, in_=yb_d[:])
        for m in range(MI):
            nc.sync.dma_start(out=en[:, m, :], in_=en_d[m])
            nc.sync.dma_start(out=ef[:, m, :], in_=ef_d[m])
        nc.sync.dma_start(out=hp, in_=hp_d[:])
        nc.sync.dma_start(out=zc, in_=zc_d[:])

        for m in range(MI):
            t1 = acc.tile([P, JB], f32, tag="t1")
            t2 = acc.tile([P, JB], f32, tag="t2")
            for j in range(JB):
                pt = psum.tile([P, JW], f32, tag="pt")
                for k in range(KC):
                    nc.tensor.matmul(
                        pt,
                        ebt[:, k, m * P:(m + 1) * P],
                        et[:, k, j * JW:(j + 1) * JW],
                        start=(k == 0),
                        stop=(k == KC - 1),
                    )
                # expS = exp(0.25*cos + 0.25); t1[:, j] = row-sum of expS
                es = work.tile([P, JW], bf16, tag="es")
                nc.scalar.activation(
                    es, pt, AF.Exp, bias=b025, scale=0.25,
                    accum_out=t1[:, j:j + 1],
                )
                # same-label mask for this j tile
                sm = work.tile([P, JW], bf16, tag="sm")
                nc.vector.tensor_scalar(
                    sm, yt[:, j * JW:(j + 1) * JW], yb[:, m:m + 1], None,
                    op0=OP.is_equal,
                )
                # t2[:, j] = row-sum(same * expS)
                mm = work.tile([P, JW], bf16, tag="mm")
                nc.vector.scalar_tensor_tensor(
                    mm, sm, 1.0, es, op0=OP.mult, op1=OP.mult,
                    accum_out=t2[:, j:j + 1],
                )
            # ns = sum_j(t1[:, j] - t2[:, j]) = T1 - T2 in one fused op
            d8 = acc.tile([P, JB], f32, tag="d8")
            ns = acc.tile([P, 1], f32, tag="ns")
            nc.vector.scalar_tensor_tensor(
                d8, t1, 1.0, t2, op0=OP.mult, op1=OP.subtract, accum_out=ns,
            )
            # pos dot: row-wise <e_i, e_firstpos(i)>
            pdo = work.tile([P, D], bf16, tag="pdo")
            pd = acc.tile([P, 1], f32, tag="pd")
            nc.vector.scalar_tensor_tensor(
                pdo, en[:, m, :], 1.0, ef[:, m, :],
                op0=OP.mult, op1=OP.mult, accum_out=pd,
            )
            # posS = 0.25*pd + 0.25 ; pos_exp = exp(posS)
            nc.vector.tensor_scalar(
                ps[:, m:m + 1], pd, 0.25, 0.25, op0=OP.mult, op1=OP.add,
            )
            pe = acc.tile([P, 1], f32, tag="pe")
            nc.scalar.activation(pe, pd, AF.Exp, bias=b025, scale=0.25)
            # total = ns + hp*(pos_exp - 1) + (zero_count + 1)
            pm1 = acc.tile([P, 1], f32, tag="pm1")
            nc.vector.tensor_scalar(pm1, pe, -1.0, None, op0=OP.add)
            hpe = acc.tile([P, 1], f32, tag="hpe")
            nc.vector.tensor_mul(hpe, pm1, hp[:, m:m + 1])
            s1 = acc.tile([P, 1], f32, tag="s1")
            nc.vector.tensor_add(s1, ns, hpe)
            nc.vector.tensor_add(tot[:, m:m + 1], s1, zc[:, m:m + 1])
        # all Ln at the end (single table-set use), then rowloss
        nc.vector.tensor_mul(pv, ps, hp)
        nc.scalar.activation(lse, tot, AF.Ln)
        nc.vector.tensor_sub(rl, lse, pv)
        nc.sync.dma_start(out=out_d[:, :], in_=rl)

    nc.compile()
    return nc


def _get_program():
    if "nc" not in _CACHE:
        _CACHE["nc"] = _build_program()
    return _CACHE["nc"]


def _host_prep(layer_embeds, y_true):
    E = np.asarray(layer_embeds, dtype=np.float32)
    y = np.asarray(y_true).astype(np.int32)

    norms = np.maximum(np.linalg.norm(E, axis=1), EPS).astype(np.float32)
    Eh = (E / norms[:, None]).astype(BF16)

    EhT = np.ascontiguousarray(Eh.T)                  # [D, N] bf16
    et = EhT.reshape(KC, P, N)                        # shared by all cores

    same = y[:, None] == y[None, :]
    nsame = same.sum(1)
    haspos = nsame > 1
    np.fill_diagonal(same, False)
    fp = np.argmax(same, axis=1)                      # first positive (j order)
    nneg = (N - nsame).astype(np.float32)
    zc1 = (2 * N - 1) - nneg                          # zero-pad count + 1

    yt = np.ascontiguousarray(np.broadcast_to(y.astype(BF16)[None, :], (P, N)))

    in_maps = []
    for c in range(NCORES):
        r0, r1 = c * R, (c + 1) * R
        in_maps.append({
            "et": et,
            "ebt": np.ascontiguousarray(et[:, :, r0:r1]),
            "yt": yt,
            "yb": np.ascontiguousarray(y[r0:r1].astype(np.float32).reshape(MI, P).T),
            "en": np.ascontiguousarray(Eh[r0:r1].reshape(MI, P, D)),
            "ef": np.ascontiguousarray(Eh[fp[r0:r1]].reshape(MI, P, D)),
            "hp": np.ascontiguousarray(
                haspos[r0:r1].astype(np.float32).reshape(MI, P).T),
            "zc": np.ascontiguousarray(zc1[r0:r1].reshape(MI, P).T),
        })
    return in_maps


def _install_ntff_shim():
    """Provide antenv.axon_hooks (absent in this image) so trace=True works."""
    import importlib
    import types
    try:
        importlib.import_module("antenv.axon_hooks")
        return
    except ImportError:
        pass
    try:
        import antenv
        from trn_agent_boot.trn_boot import _ntff_profile_via_ctypes

        hook = _ntff_profile_via_ctypes("/opt/axon/libaxon_pjrt.so")
        mod = types.ModuleType("antenv.axon_hooks")
        mod._hook = hook
        mod.get_axon_ntff_profile_hook = lambda: mod._hook
        mod.set_axon_ntff_profile_hook = lambda h: setattr(mod, "_hook", h)
        sys.modules["antenv.axon_hooks"] = mod
        antenv.axon_hooks = mod
    except Exception as e:  # profiling is best-effort
        print(f"ntff shim failed: {e}")


def kernel(layer_embeds, y_true, _trace=False):
    if _trace:
        _install_ntff_shim()
    nc = _get_program()
    in_maps = _host_prep(layer_embeds, y_true)
    res = run_bass_kernel_spmd(
        nc, in_maps, core_ids=list(range(NCORES)), trace=_trace,
    )
    total = 0.0
    for r in res.results:
        total += np.asarray(r["partial"], dtype=np.float64).sum()
    loss = np.float32(total / N)
    if _trace:
        return loss, res
    return loss


# revision 8
# speedup vs baseline: 1.5559x; 1.5559x over previous
"""Trainium2 Bass kernel for the contrastive loss problem.

Math reformulation of the reference (no [N, 2N-1] scatter needed):
  lse_i = log( exp(pos_val_i) + sum_{j in neg} exp(S_ij) + (2N-2-num_neg_i) )
  loss  = mean_i (lse_i - pos_val_i)
with S = (cos + 1) * 0.25, cos from row-normalized embeddings.

Sharding: data parallel over rows. Each of the 8 cores computes its
[512, 4096] block of exp(S) via bf16 matmul against the full normalized
embedding matrix (d-major in SBUF), does the label-masked row sums on
ScalarE/VectorE, and returns 512 per-row losses. Host does the final sum.

Host-side prep (cheap, O(N*D) or label-only): row norms, bf16 cast,
transpose to d-major, first-positive index per row (pure label metadata),
and the negatives count.
"""

import sys

sys.path.insert(0, "/opt/trn_rl_repo")

from contextlib import ExitStack

import ml_dtypes
import numpy as np

import concourse.bacc as bacc
import concourse.tile as tile
from concourse import mybir
from concourse.bass_utils import run_bass_kernel_spmd

N, D = 4096, 1024
NCORES = 8
R = N // NCORES            # 512 rows per core
P = 128                    # partitions
MI = R // P                # 4 row chunks per core
KC = D // P                # 8 contraction chunks
JW = 512                   # j tile width (one PSUM bank)
JB = N // JW               # 8 j tiles
EPS = 1e-8
BF16 = ml_dtypes.bfloat16
FP8 = ml_dtypes.float8_e4m3
SCALE = 16.0

_CACHE = {}


def _build_program():
    nc = bacc.Bacc("TRN2", target_bir_lowering=False, debug=False)
    f32, bf16, fp8 = mybir.dt.float32, mybir.dt.bfloat16, mybir.dt.float8e4
    AF = mybir.ActivationFunctionType
    OP = mybir.AluOpType

    et_d = nc.dram_tensor("et", [KC, P, N], fp8, kind="ExternalInput")
    ebt_d = nc.dram_tensor("ebt", [KC, P, R], fp8, kind="ExternalInput")
    yt_d = nc.dram_tensor("yt", [P, N], bf16, kind="ExternalInput")
    yb_d = nc.dram_tensor("yb", [P, MI], f32, kind="ExternalInput")
    en_d = nc.dram_tensor("en", [MI, P, D], bf16, kind="ExternalInput")
    ef_d = nc.dram_tensor("ef", [MI, P, D], bf16, kind="ExternalInput")
    hp_d = nc.dram_tensor("hp", [P, MI], f32, kind="ExternalInput")
    zc_d = nc.dram_tensor("zc", [P, MI], f32, kind="ExternalInput")
    out_d = nc.dram_tensor("partial", [P, MI], f32, kind="ExternalOutput")

    with tile.TileContext(nc) as tc, ExitStack() as ctx:
        const = ctx.enter_context(tc.tile_pool(name="const", bufs=1))
        psum = ctx.enter_context(tc.tile_pool(name="psum", bufs=4, space="PSUM"))
        work = ctx.enter_context(tc.tile_pool(name="work", bufs=3))
        acc = ctx.enter_context(tc.tile_pool(name="acc", bufs=2))

        et = const.tile([P, KC, N], fp8, tag="et")
        ebt = const.tile([P, KC, R], fp8, tag="ebt")
        yt = const.tile([P, N], bf16, tag="yt")
        yb = const.tile([P, MI], f32, tag="yb")
        en = const.tile([P, MI, D], bf16, tag="en")
        ef = const.tile([P, MI, D], bf16, tag="ef")
        hp = const.tile([P, MI], f32, tag="hp")
        zc = const.tile([P, MI], f32, tag="zc")
        dummy = const.tile([P, 1], f32, tag="dummy")
        b025 = const.tile([P, 1], f32, tag="b025")
        nc.vector.memset(b025, 0.25)
        tot = const.tile([P, MI], f32, tag="tot")
        ps = const.tile([P, MI], f32, tag="ps")
        pv = const.tile([P, MI], f32, tag="pv")
        lse = const.tile([P, MI], f32, tag="lse")
        rl = const.tile([P, MI], f32, tag="rl")

        for k in range(KC):
            nc.sync.dma_start(out=et[:, k, :], in_=et_d[k])
            nc.sync.dma_start(out=ebt[:, k, :], in_=ebt_d[k])
        nc.sync.dma_start(out=yt, in_=yt_d[:])
        nc.sync.dma_start(out=yb, in_=yb_d[:])
        for m in range(MI):
            nc.sync.dma_start(out=en[:, m, :], in_=en_d[m])
            nc.sync.dma_start(out=ef[:, m, :], in_=ef_d[m])
        nc.sync.dma_start(out=hp, in_=hp_d[:])
        nc.sync.dma_start(out=zc, in_=zc_d[:])

        for m in range(MI):
            t1 = acc.tile([P, JB], f32, tag="t1")
            t2 = acc.tile([P, JB], f32, tag="t2")
            for j in range(JB):
                pt = psum.tile([P, JW], f32, tag="pt")
                for k2 in range(KC // 2):
                    nc.tensor.matmul(
                        pt,
                        ebt[:, 2 * k2:2 * k2 + 2, m * P:(m + 1) * P],
                        et[:, 2 * k2:2 * k2 + 2, j * JW:(j + 1) * JW],
                        start=(k2 == 0),
                        stop=(k2 == KC // 2 - 1),
                        perf_mode=mybir.MatmulPerfMode.DoubleRow,
                    )
                # expS = exp(0.25*cos + 0.25); t1[:, j] = row-sum of expS
                es = work.tile([P, JW], bf16, tag="es")
                nc.scalar.activation(
                    es, pt, AF.Exp, bias=b025, scale=0.25 / (SCALE * SCALE),
                    accum_out=t1[:, j:j + 1],
                )
                # t2[:, j] = row-sum((y == y_row) * expS) in one fused op
                mm = work.tile([P, JW], bf16, tag="mm")
                nc.vector.scalar_tensor_tensor(
                    mm, yt[:, j * JW:(j + 1) * JW], yb[:, m:m + 1], es,
                    op0=OP.is_equal, op1=OP.mult,
                    accum_out=t2[:, j:j + 1],
                )
            # ns = sum_j(t1[:, j] - t2[:, j]) = T1 - T2 in one fused op
            d8 = acc.tile([P, JB], f32, tag="d8")
            ns = acc.tile([P, 1], f32, tag="ns")
            nc.vector.scalar_tensor_tensor(
                d8, t1, 1.0, t2, op0=OP.mult, op1=OP.subtract, accum_out=ns,
            )
            # pos dot: row-wise <e_i, e_firstpos(i)>
            pdo = work.tile([P, D], bf16, tag="pdo")
            pd = acc.tile([P, 1], f32, tag="pd")
            nc.vector.scalar_tensor_tensor(
                pdo, en[:, m, :], 1.0, ef[:, m, :],
                op0=OP.mult, op1=OP.mult, accum_out=pd,
            )
            # posS = 0.25*pd + 0.25 ; pos_exp = exp(posS)
            nc.vector.tensor_scalar(
                ps[:, m:m + 1], pd, 0.25, 0.25, op0=OP.mult, op1=OP.add,
            )
            pe = acc.tile([P, 1], f32, tag="pe")
            nc.scalar.activation(pe, pd, AF.Exp, bias=b025, scale=0.25)
            # total = ns + hp*(pos_exp - 1) + (zero_count + 1)
            pm1 = acc.tile([P, 1], f32, tag="pm1")
            nc.vector.tensor_scalar(pm1, pe, -1.0, None, op0=OP.add)
            hpe = acc.tile([P, 1], f32, tag="hpe")
            nc.vector.tensor_mul(hpe, pm1, hp[:, m:m + 1])
            s1 = acc.tile([P, 1], f32, tag="s1")
            nc.vector.tensor_add(s1, ns, hpe)
            nc.vector.tensor_add(tot[:, m:m + 1], s1, zc[:, m:m + 1])
        # all Ln at the end (single table-set use), then rowloss
        nc.vector.tensor_mul(pv, ps, hp)
        nc.scalar.activation(lse, tot, AF.Ln)
        nc.vector.tensor_sub(rl, lse, pv)
        nc.sync.dma_start(out=out_d[:, :], in_=rl)

    nc.compile()
    return nc


def _get_program():
    if "nc" not in _CACHE:
        _CACHE["nc"] = _build_program()
    return _CACHE["nc"]


def _host_prep(layer_embeds, y_true):
    E = np.asarray(layer_embeds, dtype=np.float32)
    y = np.asarray(y_true).astype(np.int32)

    norms = np.maximum(np.linalg.norm(E, axis=1), EPS).astype(np.float32)
    Ehf = E / norms[:, None]
    Eh = Ehf.astype(BF16)
    Eh8 = (Ehf * SCALE).astype(FP8)

    EhT = np.ascontiguousarray(Eh8.T)                 # [D, N] fp8, prescaled
    et = EhT.reshape(KC, P, N)                        # shared by all cores

    same = y[:, None] == y[None, :]
    nsame = same.sum(1)
    haspos = nsame > 1
    np.fill_diagonal(same, False)
    fp = np.argmax(same, axis=1)                      # first positive (j order)
    nneg = (N - nsame).astype(np.float32)
    zc1 = (2 * N - 1) - nneg                          # zero-pad count + 1

    yt = np.ascontiguousarray(np.broadcast_to(y.astype(BF16)[None, :], (P, N)))

    in_maps = []
    for c in range(NCORES):
        r0, r1 = c * R, (c + 1) * R
        in_maps.append({
            "et": et,
            "ebt": np.ascontiguousarray(et[:, :, r0:r1]),
            "yt": yt,
            "yb": np.ascontiguousarray(y[r0:r1].astype(np.float32).reshape(MI, P).T),
            "en": np.ascontiguousarray(Eh[r0:r1].reshape(MI, P, D)),
            "ef": np.ascontiguousarray(Eh[fp[r0:r1]].reshape(MI, P, D)),
            "hp": np.ascontiguousarray(
                haspos[r0:r1].astype(np.float32).reshape(MI, P).T),
            "zc": np.ascontiguousarray(zc1[r0:r1].reshape(MI, P).T),
        })
    return in_maps


def _install_ntff_shim():
    """Provide antenv.axon_hooks (absent in this image) so trace=True works."""
    import importlib
    import types
    try:
        importlib.import_module("antenv.axon_hooks")
        return
    except ImportError:
        pass
    try:
        import antenv
        from trn_agent_boot.trn_boot import _ntff_profile_via_ctypes

        hook = _ntff_profile_via_ctypes("/opt/axon/libaxon_pjrt.so")
        mod = types.ModuleType("antenv.axon_hooks")
        mod._hook = hook
        mod.get_axon_ntff_profile_hook = lambda: mod._hook
        mod.set_axon_ntff_profile_hook = lambda h: setattr(mod, "_hook", h)
        sys.modules["antenv.axon_hooks"] = mod
        antenv.axon_hooks = mod
    except Exception as e:  # profiling is best-effort
        print(f"ntff shim failed: {e}")


def kernel(layer_embeds, y_true, _trace=False):
    if _trace:
        _install_ntff_shim()
    nc = _get_program()
    in_maps = _host_prep(layer_embeds, y_true)
    res = run_bass_kernel_spmd(
        nc, in_maps, core_ids=list(range(NCORES)), trace=_trace,
    )
    total = 0.0
    for r in res.results:
        total += np.asarray(r["partial"], dtype=np.float64).sum()
    loss = np.float32(total / N)
    if _trace:
        return loss, res
    return loss
